# revision 31
# baseline (speedup 1.0000x reference)
"""Gated multi-head self-attention on 8 Trainium2 NeuronCores.

Sharding: batch (B=2) x head-groups (4 groups of 4 heads) -> 8 cores.
Each core computes, for its batch b and its 4 heads (2 pairs):
    partial_outT[e, t] = sum_h gate[h] * (softmax(Q_h K_h^T / 8) (V_h + bv_h) Wo_h + bo_h)^T
The host sums the 4 head-group partials per batch and transposes.

V2 dataflow (evidence from baseline ntff trace):
  - QKV projections in fp8e4 (weights pre-scaled x32 on host, undone in the
    psum->sbuf copy, to dodge fp8 subnormals). DoubleRow is NOT usable here:
    walrus's s3d3_mm_valid_dst_partition check forces DR outputs to psum
    partition base 0 (each DR logical column uses 2 physical PE columns).
  - scores bf16 row-split quadrant pairs; PV/rowsum bf16 col-split pairs
    (the ntff trace proves paired tile_position matmuls run concurrently,
    which matches DR throughput without the partition-0 restriction).
  - exp on ACT only -> bf16 ex feeds PV + rowsum consistently.
  - rowsum via paired ones-matmuls accumulated on PE (replaces the serial
    DVE chain-add of the baseline).
  - reciprocal_approx_fast; R broadcast + bv/ctx add on GpSimd.
  - qk/v/o biases folded into DVE tensor_scalar psum->sbuf copies.
  - st-loop software-pipelined by one stage so PE never waits on ACT.
"""

import numpy as np
import ml_dtypes
from contextlib import ExitStack

import concourse.bass as bass
import concourse.tile as tile
from concourse import bacc, mybir
from concourse import bass_utils

E, H, D = 1024, 16, 64
B, T = 2, 2048
NCORES = 8
P = 128
TC = 512          # t-chunk (PSUM bank = 512 fp32)
NTC = T // TC     # 4 t-chunks
NST = T // P      # 16 s-tiles of 128
NSP = NST // 2    # 8 s-tile pairs (DoubleRow k=256)
NEC = E // P      # 8 e-chunks
WS = 32.0         # weight prescale (fp8 subnormal dodge)

F32 = mybir.dt.float32
BF16 = mybir.dt.bfloat16
F8 = mybir.dt.float8e4
DR = mybir.MatmulPerfMode.DoubleRow
EXP = mybir.ActivationFunctionType.Exp
MUL = mybir.AluOpType.mult
ADD = mybir.AluOpType.add


def build_kernel():
    nc = bacc.Bacc("TRN2", target_bir_lowering=False, debug=False,
                   num_devices=NCORES)
    hT8 = nc.dram_tensor("hT8", [P, NEC, T], F8, kind="ExternalInput").ap()
    wq8 = nc.dram_tensor("wq8", [P, NEC, 256], F8, kind="ExternalInput").ap()
    wk8 = nc.dram_tensor("wk8", [P, NEC, 256], F8, kind="ExternalInput").ap()
    wv8 = nc.dram_tensor("wv8", [P, NEC, 256], F8, kind="ExternalInput").ap()
    wo16 = nc.dram_tensor("wo16", [P, 2, E], BF16, kind="ExternalInput").ap()
    bq2 = nc.dram_tensor("bq2", [P, 2], F32, kind="ExternalInput").ap()
    bk2 = nc.dram_tensor("bk2", [P, 2], F32, kind="ExternalInput").ap()
    bv2 = nc.dram_tensor("bv2", [P, 2], F32, kind="ExternalInput").ap()
    bo8 = nc.dram_tensor("bo8", [P, NEC], F32, kind="ExternalInput").ap()
    ones8 = nc.dram_tensor("ones8", [P, 1], F32, kind="ExternalInput").ap()
    sel = nc.dram_tensor("sel", [1, 2 * P], F32, kind="ExternalInput").ap()
    outT = nc.dram_tensor("outT", [E, T], F32, kind="ExternalOutput").ap()

    with tile.TileContext(nc) as tc:
        with ExitStack() as ctx:
            persist = ctx.enter_context(tc.tile_pool(name="persist", bufs=1))
            expool = ctx.enter_context(tc.tile_pool(name="expool", bufs=6))
            work = ctx.enter_context(tc.tile_pool(name="work", bufs=4))
            rspool = ctx.enter_context(tc.tile_pool(name="rspool", bufs=4))
            opool = ctx.enter_context(tc.tile_pool(name="opool", bufs=3))
            ps_s = ctx.enter_context(tc.tile_pool(name="ps_s", bufs=2, space="PSUM"))
            ps_a = ctx.enter_context(tc.tile_pool(name="ps_a", bufs=2, space="PSUM"))
            ps_b = ctx.enter_context(tc.tile_pool(name="ps_b", bufs=2, space="PSUM"))

            # ---- persistent SBUF ----
            hT_sb = persist.tile([P, NEC, T], F8, tag="hT")
            wq_sb = persist.tile([P, NEC, 256], F8, tag="wq")
            wk_sb = persist.tile([P, NEC, 256], F8, tag="wk")
            wv_sb = persist.tile([P, NEC, 256], F8, tag="wv")
            wo_sb = persist.tile([P, 2, E], BF16, tag="wo")
            bq_sb = persist.tile([P, 2], F32, tag="bq")
            bk_sb = persist.tile([P, 2], F32, tag="bk")
            bv_sb = persist.tile([P, 2], F32, tag="bv")
            bo_sb = persist.tile([P, NEC], F32, tag="bo")
            on_sb = persist.tile([P, 1], F32, tag="ones")
            sel_sb = persist.tile([1, 2 * P], F32, tag="sel")
            QT = persist.tile([P, 2, T], BF16, tag="QT")
            KT = persist.tile([P, 2, T], BF16, tag="KT")
            Vt = persist.tile([P, NST, 256], BF16, tag="Vt")
            ctx_sb = persist.tile([P, 2, T], BF16, tag="ctx")

            with nc.named_scope("load"):
                nc.sync.dma_start(wq_sb[:], wq8)
                nc.sync.dma_start(wk_sb[:], wk8)
                nc.sync.dma_start(bq_sb[:], bq2)
                nc.sync.dma_start(bk_sb[:], bk2)
                for j in range(4):
                    nc.sync.dma_start(hT_sb[:, 2 * j:2 * j + 2, :],
                                      hT8[:, 2 * j:2 * j + 2, :])
                nc.sync.dma_start(wv_sb[:], wv8)
                nc.sync.dma_start(wo_sb[:], wo16)
                nc.sync.dma_start(bv_sb[:], bv2)
                nc.sync.dma_start(bo_sb[:], bo8)
                nc.sync.dma_start(on_sb[:], ones8)
                nc.sync.dma_start(sel_sb[:], sel)

            # ---- phase 1: K/Q (pr0) -> V -> K/Q (pr1) so exp starts early ----
            def kq_proj(pr, w_sb, b_sb, dst):
                for tch in range(NTC):
                    t0 = tch * TC
                    qk_ps = ps_a.tile([P, TC], F32, tag="ps_a")
                    for ec in range(NEC):
                        nc.tensor.matmul(
                            qk_ps[:],
                            w_sb[:, ec, pr * P:(pr + 1) * P],
                            hT_sb[:, ec, t0:t0 + TC],
                            start=(ec == 0), stop=(ec == NEC - 1))
                    nc.vector.tensor_scalar(
                        dst[:, pr, t0:t0 + TC], qk_ps[:],
                        1.0 / WS, b_sb[:, pr:pr + 1], MUL, ADD)

            with nc.named_scope("qkv"):
                kq_proj(0, wk_sb, bk_sb, KT)
                kq_proj(0, wq_sb, bq_sb, QT)
                # V projection: out [t-tile(128), d4(256)]
                for tt in range(NST):
                    v_ps = ps_b.tile([P, TC], F32, tag="ps_b")
                    for ec in range(NEC):
                        nc.tensor.matmul(
                            v_ps[:, 0:256],
                            hT_sb[:, ec, tt * P:(tt + 1) * P],
                            wv_sb[:, ec, :],
                            start=(ec == 0), stop=(ec == NEC - 1))
                    nc.vector.tensor_scalar(
                        Vt[:, tt, :], v_ps[:, 0:256], 1.0 / WS, None, MUL)
                kq_proj(1, wk_sb, bk_sb, KT)
                kq_proj(1, wq_sb, bq_sb, QT)

            # ---- phase 2+3: attention + interleaved output projection ----
            for tch in range(NTC):
                t0 = tch * TC
                with nc.named_scope("attn"):
                    for pr in range(2):
                        pctx = ps_a.tile([P, TC], F32, tag="ps_a")
                        rsA = rspool.tile([P, TC], F32, tag="rsA")
                        rsB = rspool.tile([P, TC], F32, tag="rsB")
                        ex_tiles = []

                        def scores_st(st):
                            s0 = st * P
                            ex = expool.tile([P, 2 * TC], BF16, tag="ex")
                            pss = ps_s.tile([P, 2 * TC], F32, tag="ps_s")
                            nc.tensor.matmul(
                                pss[:, :TC], KT[0:64, pr, s0:s0 + P],
                                QT[0:64, pr, t0:t0 + TC],
                                start=True, stop=True)
                            nc.tensor.matmul(
                                pss[:, TC:], KT[64:P, pr, s0:s0 + P],
                                QT[64:P, pr, t0:t0 + TC],
                                start=True, stop=True)
                            nc.scalar.activation(ex[:], pss[:], EXP, scale=0.125)
                            return ex

                        def pv_rs_st(st, ex):
                            for hh in range(2):
                                c0 = pr * P + hh * 64
                                nc.tensor.matmul(
                                    pctx[hh * 64:(hh + 1) * 64, :],
                                    Vt[:, st, c0:c0 + 64],
                                    ex[:, hh * TC:(hh + 1) * TC],
                                    start=(st == 0), stop=(st == NST - 1),
                                    tile_position=(0, hh * 64),
                                    skip_group_check=True)
                            # fp32 rowsum chains: head A on DVE, head B on Pool
                            if st == 0:
                                nc.vector.tensor_copy(rsA[:], ex[:, :TC])
                                nc.gpsimd.tensor_copy(rsB[:], ex[:, TC:])
                            else:
                                nc.vector.tensor_add(rsA[:], rsA[:], ex[:, :TC])
                                nc.gpsimd.tensor_add(rsB[:], rsB[:], ex[:, TC:])

                        # software pipeline: PV/rs lag scores by two stages
                        for st in range(NST):
                            ex_tiles.append(scores_st(st))
                            if st > 1:
                                pv_rs_st(st - 2, ex_tiles[st - 2])
                        pv_rs_st(NST - 2, ex_tiles[NST - 2])
                        pv_rs_st(NST - 1, ex_tiles[NST - 1])

                        # denominators: paired fp32 ones-matmuls over rs tiles
                        den_ps = ps_b.tile([P, TC], F32, tag="ps_b")
                        for hh, rstile in ((0, rsA), (1, rsB)):
                            nc.tensor.matmul(
                                den_ps[hh * 64:hh * 64 + 1, :], on_sb[:],
                                rstile[:],
                                start=True, stop=True,
                                tile_position=(0, hh * 64),
                                skip_group_check=True)
                        # normalize: ctx = pctx / rowsum + bv
                        rcps = []
                        for hh in range(2):
                            rcp = work.tile([1, TC], F32, tag="rcp")
                            nc.vector.reciprocal_approx_fast(
                                rcp[:], den_ps[hh * 64:hh * 64 + 1, :])
                            rcps.append(rcp)
                        pR = ps_b.tile([P, TC], F32, tag="ps_b")
                        nc.tensor.matmul(pR[:], sel_sb[:, 0:P], rcps[0][:],
                                         start=True, stop=False)
                        nc.tensor.matmul(pR[:], sel_sb[:, P:2 * P], rcps[1][:],
                                         start=False, stop=True)
                        R_sb = work.tile([P, TC], F32, tag="Rb")
                        nc.vector.tensor_copy(R_sb[:], pR[:])
                        tmp = work.tile([P, TC], BF16, tag="tmp")
                        nc.vector.tensor_tensor(tmp[:], pctx[:], R_sb[:], MUL)
                        nc.vector.tensor_scalar(
                            ctx_sb[:, pr, t0:t0 + TC], tmp[:],
                            bv_sb[:, pr:pr + 1], None, ADD)

                with nc.named_scope("outproj"):
                    for et in range(NEC):
                        po = ps_b.tile([P, TC], F32, tag="ps_b")
                        for pr in range(2):
                            nc.tensor.matmul(
                                po[:], wo_sb[:, pr, et * P:(et + 1) * P],
                                ctx_sb[:, pr, t0:t0 + TC],
                                start=(pr == 0), stop=(pr == 1))
                        o_sb = opool.tile([P, TC], F32, tag="o")
                        nc.vector.tensor_scalar(
                            o_sb[:], po[:], bo_sb[:, et:et + 1], None, ADD)
                        nc.sync.dma_start(
                            outT[et * P:(et + 1) * P, t0:t0 + TC], o_sb[:])
    nc.compile()
    return nc


_NC = None


def _get_nc():
    global _NC
    if _NC is None:
        _NC = build_kernel()
    return _NC


def make_in_maps(hidden_states, Wq, bq, Wk, bk, Wv, bv, Wo, bo, gate):
    f = np.float32
    f8 = ml_dtypes.float8_e4m3
    hidden_states = np.asarray(hidden_states, f)
    Wq, bq = np.asarray(Wq, f), np.asarray(bq, f)
    Wk, bk = np.asarray(Wk, f), np.asarray(bk, f)
    Wv, bv = np.asarray(Wv, f), np.asarray(bv, f)
    Wo, bo = np.asarray(Wo, f), np.asarray(bo, f)
    gate = np.asarray(gate, f)

    # hT8 per batch: [128, NEC, T] fp8
    hT8_b = []
    for b in range(B):
        ht = np.ascontiguousarray(
            hidden_states[b].T.reshape(NEC, P, T).transpose(1, 0, 2))
        hT8_b.append(ht.astype(f8))

    ones8 = np.ones((P, 1), f)
    sel_np = np.zeros((1, 2 * P), f)
    sel_np[0, 0:64] = 1.0          # head-A rows of pctx
    sel_np[0, P + 64:2 * P] = 1.0  # head-B rows of pctx

    def pack_w(W, hs):
        # [1024, 256] = concat over 4 heads, x32, -> [128, NEC, 256] fp8
        wcat = np.concatenate([W[h] for h in hs], axis=1) * WS
        return np.ascontiguousarray(
            wcat.reshape(NEC, P, 256).transpose(1, 0, 2)).astype(f8)

    in_maps = []
    for core in range(NCORES):
        b, hg = divmod(core, 4)
        hs = [4 * hg + i for i in range(4)]
        wo16 = np.empty((P, 2, E), ml_dtypes.bfloat16)
        bq2 = np.empty((P, 2), f)
        bk2 = np.empty((P, 2), f)
        bv2 = np.empty((P, 2), f)
        for pr in range(2):
            h0, h1 = hs[2 * pr], hs[2 * pr + 1]
            wo16[:, pr, :] = np.concatenate(
                [gate[h0] * Wo[h0], gate[h1] * Wo[h1]], axis=0).astype(
                    ml_dtypes.bfloat16)
            bq2[:, pr] = np.concatenate([bq[h0], bq[h1]])
            bk2[:, pr] = np.concatenate([bk[h0], bk[h1]])
            bv2[:, pr] = np.concatenate([bv[h0], bv[h1]])
        bo_f = sum(gate[h] * bo[h] for h in hs)          # [1024]
        bo8 = np.ascontiguousarray(bo_f.reshape(NEC, P).T)  # [128, NEC]
        in_maps.append(dict(
            hT8=hT8_b[b],
            wq8=pack_w(Wq, hs), wk8=pack_w(Wk, hs), wv8=pack_w(Wv, hs),
            wo16=np.ascontiguousarray(wo16),
            bq2=bq2, bk2=bk2, bv2=bv2, bo8=bo8, ones8=ones8, sel=sel_np,
        ))
    return in_maps


def kernel(hidden_states, Wq, bq, Wk, bk, Wv, bv, Wo, bo, gate, _trace=False,
           **run_kwargs):
    nc = _get_nc()
    in_maps = make_in_maps(hidden_states, Wq, bq, Wk, bk, Wv, bv, Wo, bo, gate)
    res = bass_utils.run_bass_kernel_spmd(
        nc, in_maps, core_ids=list(range(NCORES)), trace=_trace, **run_kwargs)
    outs = [r["outT"] for r in res.results]
    full = np.stack([
        (outs[0] + outs[1] + outs[2] + outs[3]).T,
        (outs[4] + outs[5] + outs[6] + outs[7]).T,
    ]).astype(np.float32)
    kernel.last_result = res
    return full


# revision 37
# speedup vs baseline: 1.2273x; 1.2273x over previous
"""Gated multi-head self-attention on 8 Trainium2 NeuronCores.

Sharding: batch (B=2) x head-groups (4 groups of 4 heads) -> 8 cores.
Each core computes, for its batch b and its 4 heads (2 pairs):
    partial_outT[e, t] = sum_h gate[h] * (softmax(Q_h K_h^T / 8) (V_h + bv_h) Wo_h + bo_h)^T
The host sums the 4 head-group partials per batch and transposes.

V2 dataflow (evidence from baseline ntff trace):
  - QKV projections in fp8e4 (weights pre-scaled x32 on host, undone in the
    psum->sbuf copy, to dodge fp8 subnormals). DoubleRow is NOT usable here:
    walrus's s3d3_mm_valid_dst_partition check forces DR outputs to psum
    partition base 0 (each DR logical column uses 2 physical PE columns).
  - scores bf16 row-split quadrant pairs; PV/rowsum bf16 col-split pairs
    (the ntff trace proves paired tile_position matmuls run concurrently,
    which matches DR throughput without the partition-0 restriction).
  - exp on ACT only -> bf16 ex feeds PV + rowsum consistently.
  - rowsum via paired ones-matmuls accumulated on PE (replaces the serial
    DVE chain-add of the baseline).
  - reciprocal_approx_fast; R broadcast + bv/ctx add on GpSimd.
  - qk/v/o biases folded into DVE tensor_scalar psum->sbuf copies.
  - st-loop software-pipelined by one stage so PE never waits on ACT.
"""

import numpy as np
import ml_dtypes
from contextlib import ExitStack

import concourse.bass as bass
import concourse.tile as tile
from concourse import bacc, mybir
from concourse import bass_utils

E, H, D = 1024, 16, 64
B, T = 2, 2048
NCORES = 8
P = 128
TC = 512          # t-chunk (PSUM bank = 512 fp32)
NTC = T // TC     # 4 t-chunks
NST = T // P      # 16 s-tiles of 128
NSP = NST // 2    # 8 s-tile pairs (DoubleRow k=256)
NEC = E // P      # 8 e-chunks
WS = 32.0         # weight prescale (fp8 subnormal dodge)

F32 = mybir.dt.float32
BF16 = mybir.dt.bfloat16
F8 = mybir.dt.float8e4
DR = mybir.MatmulPerfMode.DoubleRow
EXP = mybir.ActivationFunctionType.Exp
MUL = mybir.AluOpType.mult
ADD = mybir.AluOpType.add


def build_kernel():
    nc = bacc.Bacc("TRN2", target_bir_lowering=False, debug=False,
                   num_devices=NCORES)
    hT8 = nc.dram_tensor("hT8", [P, NEC, T], F8, kind="ExternalInput").ap()
    wq8 = nc.dram_tensor("wq8", [P, NEC, 256], F8, kind="ExternalInput").ap()
    wk8 = nc.dram_tensor("wk8", [P, NEC, 256], F8, kind="ExternalInput").ap()
    wv8 = nc.dram_tensor("wv8", [P, NEC, 256], F8, kind="ExternalInput").ap()
    wo16 = nc.dram_tensor("wo16", [P, 2, E], BF16, kind="ExternalInput").ap()
    bq2 = nc.dram_tensor("bq2", [P, 2], F32, kind="ExternalInput").ap()
    bk2 = nc.dram_tensor("bk2", [P, 2], F32, kind="ExternalInput").ap()
    bv2 = nc.dram_tensor("bv2", [P, 2], F32, kind="ExternalInput").ap()
    bo8 = nc.dram_tensor("bo8", [P, NEC], F32, kind="ExternalInput").ap()
    ones8 = nc.dram_tensor("ones8", [P, 1], BF16, kind="ExternalInput").ap()
    sel = nc.dram_tensor("sel", [1, 2 * P], F32, kind="ExternalInput").ap()
    outT = nc.dram_tensor("outT", [E, T], F32, kind="ExternalOutput").ap()

    with tile.TileContext(nc) as tc:
        with ExitStack() as ctx:
            persist = ctx.enter_context(tc.tile_pool(name="persist", bufs=1))
            expool = ctx.enter_context(tc.tile_pool(name="expool", bufs=6))
            work = ctx.enter_context(tc.tile_pool(name="work", bufs=4))
            rspool = ctx.enter_context(tc.tile_pool(name="rspool", bufs=4))
            opool = ctx.enter_context(tc.tile_pool(name="opool", bufs=3))
            ps_s = ctx.enter_context(tc.tile_pool(name="ps_s", bufs=2, space="PSUM"))
            ps_a = ctx.enter_context(tc.tile_pool(name="ps_a", bufs=2, space="PSUM"))
            ps_b = ctx.enter_context(tc.tile_pool(name="ps_b", bufs=2, space="PSUM"))

            # ---- persistent SBUF ----
            hT_sb = persist.tile([P, NEC, T], F8, tag="hT")
            wq_sb = persist.tile([P, NEC, 256], F8, tag="wq")
            wk_sb = persist.tile([P, NEC, 256], F8, tag="wk")
            wv_sb = persist.tile([P, NEC, 256], F8, tag="wv")
            wo_sb = persist.tile([P, 2, E], BF16, tag="wo")
            bq_sb = persist.tile([P, 2], F32, tag="bq")
            bk_sb = persist.tile([P, 2], F32, tag="bk")
            bv_sb = persist.tile([P, 2], F32, tag="bv")
            bo_sb = persist.tile([P, NEC], F32, tag="bo")
            on_sb = persist.tile([P, 1], BF16, tag="ones")
            sel_sb = persist.tile([1, 2 * P], F32, tag="sel")
            QT = persist.tile([P, 2, T], BF16, tag="QT")
            KT = persist.tile([P, 2, T], BF16, tag="KT")
            Vt = persist.tile([P, NST, 256], BF16, tag="Vt")
            ctx_sb = persist.tile([P, 2, T], BF16, tag="ctx")

            with nc.named_scope("load"):
                nc.sync.dma_start(wq_sb[:], wq8)
                nc.sync.dma_start(wk_sb[:], wk8)
                nc.sync.dma_start(bq_sb[:], bq2)
                nc.sync.dma_start(bk_sb[:], bk2)
                for j in range(4):
                    nc.sync.dma_start(hT_sb[:, 2 * j:2 * j + 2, :],
                                      hT8[:, 2 * j:2 * j + 2, :])
                nc.sync.dma_start(wv_sb[:], wv8)
                nc.sync.dma_start(wo_sb[:], wo16)
                nc.sync.dma_start(bv_sb[:], bv2)
                nc.sync.dma_start(bo_sb[:], bo8)
                nc.sync.dma_start(on_sb[:], ones8)
                nc.sync.dma_start(sel_sb[:], sel)

            # ---- phase 1: K/Q (pr0) -> V -> K/Q (pr1) so exp starts early ----
            def kq_proj(pr, w_sb, b_sb, dst):
                for tch in range(NTC):
                    t0 = tch * TC
                    qk_ps = ps_a.tile([P, TC], F32, tag="ps_a")
                    for ec in range(NEC):
                        nc.tensor.matmul(
                            qk_ps[:],
                            w_sb[:, ec, pr * P:(pr + 1) * P],
                            hT_sb[:, ec, t0:t0 + TC],
                            start=(ec == 0), stop=(ec == NEC - 1))
                    nc.vector.tensor_scalar(
                        dst[:, pr, t0:t0 + TC], qk_ps[:],
                        1.0 / WS, b_sb[:, pr:pr + 1], MUL, ADD)

            with nc.named_scope("qkv"):
                kq_proj(0, wk_sb, bk_sb, KT)
                kq_proj(0, wq_sb, bq_sb, QT)
                # V projection: out [t-tile(128), d4(256)]
                for tt in range(NST):
                    v_ps = ps_b.tile([P, TC], F32, tag="ps_b")
                    for ec in range(NEC):
                        nc.tensor.matmul(
                            v_ps[:, 0:256],
                            hT_sb[:, ec, tt * P:(tt + 1) * P],
                            wv_sb[:, ec, :],
                            start=(ec == 0), stop=(ec == NEC - 1))
                    nc.vector.tensor_scalar(
                        Vt[:, tt, :], v_ps[:, 0:256], 1.0 / WS, None, MUL)
                kq_proj(1, wk_sb, bk_sb, KT)
                kq_proj(1, wq_sb, bq_sb, QT)

            # ---- phase 2+3: attention + interleaved output projection ----
            for tch in range(NTC):
                t0 = tch * TC
                with nc.named_scope("attn"):
                    for pr in range(2):
                        pctx = ps_a.tile([P, TC], F32, tag="ps_a")
                        rs = rspool.tile([P, 2 * TC], BF16, tag="rs")
                        ex_tiles = []

                        def scores_st(st):
                            s0 = st * P
                            ex = expool.tile([P, 2 * TC], BF16, tag="ex")
                            pss = ps_s.tile([P, 2 * TC], F32, tag="ps_s")
                            nc.tensor.matmul(
                                pss[:, :TC], KT[0:64, pr, s0:s0 + P],
                                QT[0:64, pr, t0:t0 + TC],
                                start=True, stop=True)
                            nc.tensor.matmul(
                                pss[:, TC:], KT[64:P, pr, s0:s0 + P],
                                QT[64:P, pr, t0:t0 + TC],
                                start=True, stop=True)
                            nc.scalar.activation(ex[:], pss[:], EXP, scale=0.125)
                            return ex

                        def pv_rs_st(st, ex):
                            for hh in range(2):
                                c0 = pr * P + hh * 64
                                nc.tensor.matmul(
                                    pctx[hh * 64:(hh + 1) * 64, :],
                                    Vt[:, st, c0:c0 + 64],
                                    ex[:, hh * TC:(hh + 1) * TC],
                                    start=(st == 0), stop=(st == NST - 1),
                                    tile_position=(0, hh * 64),
                                    skip_group_check=True)
                            # bf16 rowsum chain on DVE (2x mode)
                            if st == 0:
                                nc.vector.tensor_copy(rs[:], ex[:])
                            else:
                                nc.vector.tensor_add(rs[:], rs[:], ex[:])

                        # software pipeline: PV/rs lag scores by two stages
                        for st in range(NST):
                            ex_tiles.append(scores_st(st))
                            if st > 1:
                                pv_rs_st(st - 2, ex_tiles[st - 2])
                        pv_rs_st(NST - 2, ex_tiles[NST - 2])
                        pv_rs_st(NST - 1, ex_tiles[NST - 1])

                        # denominators: paired ones-matmuls over the rs tile
                        den_ps = ps_b.tile([P, TC], F32, tag="ps_b")
                        for hh in range(2):
                            nc.tensor.matmul(
                                den_ps[hh * 64:hh * 64 + 1, :], on_sb[:],
                                rs[:, hh * TC:(hh + 1) * TC],
                                start=True, stop=True,
                                tile_position=(0, hh * 64),
                                skip_group_check=True)
                        # normalize: ctx = pctx / rowsum + bv.
                        # approx recip seed + one Newton-Raphson pass with
                        # standard DVE ops (hw approx is cruder than CoreSim).
                        rcps = []
                        for hh in range(2):
                            den = den_ps[hh * 64:hh * 64 + 1, :]
                            r0 = work.tile([1, TC], F32, tag="rcp0")
                            nc.vector.reciprocal_approx_fast(r0[:], den)
                            t = work.tile([1, TC], F32, tag="rcpt")
                            nc.vector.tensor_tensor(t[:], den, r0[:], MUL)
                            nc.vector.tensor_scalar(
                                t[:], t[:], -1.0, 2.0, MUL, ADD)
                            rcp = work.tile([1, TC], F32, tag="rcp")
                            nc.vector.tensor_tensor(rcp[:], r0[:], t[:], MUL)
                            rcps.append(rcp)
                        pR = ps_b.tile([P, TC], F32, tag="ps_b")
                        nc.tensor.matmul(pR[:], sel_sb[:, 0:P], rcps[0][:],
                                         start=True, stop=False)
                        nc.tensor.matmul(pR[:], sel_sb[:, P:2 * P], rcps[1][:],
                                         start=False, stop=True)
                        R_sb = work.tile([P, TC], F32, tag="Rb")
                        nc.vector.tensor_copy(R_sb[:], pR[:])
                        tmp = work.tile([P, TC], BF16, tag="tmp")
                        nc.vector.tensor_tensor(tmp[:], pctx[:], R_sb[:], MUL)
                        nc.vector.tensor_scalar(
                            ctx_sb[:, pr, t0:t0 + TC], tmp[:],
                            bv_sb[:, pr:pr + 1], None, ADD)

                with nc.named_scope("outproj"):
                    for et in range(NEC):
                        po = ps_b.tile([P, TC], F32, tag="ps_b")
                        for pr in range(2):
                            nc.tensor.matmul(
                                po[:], wo_sb[:, pr, et * P:(et + 1) * P],
                                ctx_sb[:, pr, t0:t0 + TC],
                                start=(pr == 0), stop=(pr == 1))
                        o_sb = opool.tile([P, TC], F32, tag="o")
                        nc.vector.tensor_scalar(
                            o_sb[:], po[:], bo_sb[:, et:et + 1], None, ADD)
                        nc.sync.dma_start(
                            outT[et * P:(et + 1) * P, t0:t0 + TC], o_sb[:])
    nc.compile()
    return nc


_NC = None


def _get_nc():
    global _NC
    if _NC is None:
        _NC = build_kernel()
    return _NC


def make_in_maps(hidden_states, Wq, bq, Wk, bk, Wv, bv, Wo, bo, gate):
    f = np.float32
    f8 = ml_dtypes.float8_e4m3
    hidden_states = np.asarray(hidden_states, f)
    Wq, bq = np.asarray(Wq, f), np.asarray(bq, f)
    Wk, bk = np.asarray(Wk, f), np.asarray(bk, f)
    Wv, bv = np.asarray(Wv, f), np.asarray(bv, f)
    Wo, bo = np.asarray(Wo, f), np.asarray(bo, f)
    gate = np.asarray(gate, f)

    # hT8 per batch: [128, NEC, T] fp8
    hT8_b = []
    for b in range(B):
        ht = np.ascontiguousarray(
            hidden_states[b].T.reshape(NEC, P, T).transpose(1, 0, 2))
        hT8_b.append(ht.astype(f8))

    ones8 = np.ones((P, 1), ml_dtypes.bfloat16)
    sel_np = np.zeros((1, 2 * P), f)
    sel_np[0, 0:64] = 1.0          # head-A rows of pctx
    sel_np[0, P + 64:2 * P] = 1.0  # head-B rows of pctx

    def pack_w(W, hs):
        # [1024, 256] = concat over 4 heads, x32, -> [128, NEC, 256] fp8
        wcat = np.concatenate([W[h] for h in hs], axis=1) * WS
        return np.ascontiguousarray(
            wcat.reshape(NEC, P, 256).transpose(1, 0, 2)).astype(f8)

    in_maps = []
    for core in range(NCORES):
        b, hg = divmod(core, 4)
        hs = [4 * hg + i for i in range(4)]
        wo16 = np.empty((P, 2, E), ml_dtypes.bfloat16)
        bq2 = np.empty((P, 2), f)
        bk2 = np.empty((P, 2), f)
        bv2 = np.empty((P, 2), f)
        for pr in range(2):
            h0, h1 = hs[2 * pr], hs[2 * pr + 1]
            wo16[:, pr, :] = np.concatenate(
                [gate[h0] * Wo[h0], gate[h1] * Wo[h1]], axis=0).astype(
                    ml_dtypes.bfloat16)
            bq2[:, pr] = np.concatenate([bq[h0], bq[h1]])
            bk2[:, pr] = np.concatenate([bk[h0], bk[h1]])
            bv2[:, pr] = np.concatenate([bv[h0], bv[h1]])
        bo_f = sum(gate[h] * bo[h] for h in hs)          # [1024]
        bo8 = np.ascontiguousarray(bo_f.reshape(NEC, P).T)  # [128, NEC]
        in_maps.append(dict(
            hT8=hT8_b[b],
            wq8=pack_w(Wq, hs), wk8=pack_w(Wk, hs), wv8=pack_w(Wv, hs),
            wo16=np.ascontiguousarray(wo16),
            bq2=bq2, bk2=bk2, bv2=bv2, bo8=bo8, ones8=ones8, sel=sel_np,
        ))
    return in_maps


def kernel(hidden_states, Wq, bq, Wk, bk, Wv, bv, Wo, bo, gate, _trace=False,
           **run_kwargs):
    nc = _get_nc()
    in_maps = make_in_maps(hidden_states, Wq, bq, Wk, bk, Wv, bv, Wo, bo, gate)
    res = bass_utils.run_bass_kernel_spmd(
        nc, in_maps, core_ids=list(range(NCORES)), trace=_trace, **run_kwargs)
    outs = [r["outT"] for r in res.results]
    full = np.stack([
        (outs[0] + outs[1] + outs[2] + outs[3]).T,
        (outs[4] + outs[5] + outs[6] + outs[7]).T,
    ]).astype(np.float32)
    kernel.last_result = res
    return full
